# revision 14
# baseline (speedup 1.0000x reference)
"""Trainium2 Bass kernel for SoftPathMDDLoss — PE-bilinear, ACT-lean.

Math: per 128-step subblock (time on partitions, (path, subblock)
path-major on the free axis), with strict suffix sums zr' = U'.b and
host f64 cross-block composition.  Engine assignment from measured
TRN2 rates:
  - host pre-casts the input to fp16 (halves DMA-in to 16 MiB/core).
  - Ln(1+r) ~= r*(1 - r/2): 2-op fp16 polynomial on DVE at [128,4096]
    granularity where the 4x/2x perf modes engage (r^3/3 < 3e-6 is far
    below the fp16 quantisation already present).  ACT therefore runs
    only the two Exp passes (bf16 out, no cast passes anywhere).
  - sig (per-block sum of b) is not exported: the host replicates the
    fp16 polynomial + f32 partition sum bit-closely (~1e-7), which is
    exact enough for the log-domain corrections.
  - per-unit row exports (emr127 / er127 / z127) collect in bf16 arena
    tiles (3 bufs) and DMA once per 8 units; ~85 dma_starts total.
  - PE: 4 passes (U'b cumsum, Sm = L emr, rotating one-hot partition
    sums of Z = er*Sm -> gi and of er -> ser), 512-col matmuls, fp16/
    bf16 moving at 1 cyc/row; sustained-fed PE holds the 2.4 GHz
    p-state.
Pipeline: per 1024-col iter k issue U'b(k)+exps(k), rot(k-3), Sm/Z(k-1);
PSUM = zq[1024]x2 + smt[1024] + rgi + rser = 8 banks.
Measured: ~192.6 us on 8 cores (baseline 226.9), rel err 2.5e-3.
"""
import numpy as np

import concourse.bacc as bacc
import concourse.mybir as mybir
import concourse.tile as tile
from concourse.bass_utils import run_bass_kernel_spmd

T, B = 65536, 1024
NCORES = 8
PL = 128                  # paths per core (unit u <-> path u)
SB = 128                  # subblock length = partitions
Q = T // SB               # 512 subblocks per path
F = PL * Q                # 65536 free columns per core
ST = 512                  # unit width = one psum bank
ZQW = 1024                # zq tile width (2 units, one iter)
XTW = 8192                # input tile width
ARW = 4096                # arena width (8 units = 4 iters)
NGR = F // ARW            # 16 arena groups
NIT = F // ZQW            # 64 iters
BETA = 40.0
THRESHOLD = 0.12
MDD_LAMBDA = 5.0

F32 = mybir.dt.float32
BF16 = mybir.dt.bfloat16
FP16 = mybir.dt.float16
AF = mybir.ActivationFunctionType
ALU = mybir.AluOpType

_built = {}


def build():
    if "nc" in _built:
        return _built["nc"]
    nc = bacc.Bacc("TRN2", target_bir_lowering=False, debug=False)
    xt = nc.dram_tensor("xt", [SB, F], FP16, kind="ExternalInput").ap()
    cu = nc.dram_tensor("cu", [SB, SB], F32, kind="ExternalInput").ap()
    cl = nc.dram_tensor("cl", [SB, SB], F32, kind="ExternalInput").ap()
    cz = nc.dram_tensor("cz", [SB, 255], F32, kind="ExternalInput").ap()
    o_mr = nc.dram_tensor("mr", [NGR, ARW], BF16, kind="ExternalOutput").ap()
    o_er = nc.dram_tensor("er", [NGR, ARW], BF16, kind="ExternalOutput").ap()
    o_zp = nc.dram_tensor("zp", [NGR, ARW], BF16, kind="ExternalOutput").ap()
    o_sr = nc.dram_tensor("sr", [128, ST], F32, kind="ExternalOutput").ap()
    o_gi = nc.dram_tensor("gi", [128, ST], F32, kind="ExternalOutput").ap()

    with tile.TileContext(nc) as tc:
        with tc.tile_pool(name="io", bufs=2) as iop, \
             tc.tile_pool(name="bb", bufs=2) as bbp, \
             tc.tile_pool(name="ar", bufs=3) as arp, \
             tc.tile_pool(name="st", bufs=1) as sp, \
             tc.tile_pool(name="ps", bufs=1,
                          space=bacc.bass.MemorySpace.PSUM) as pp:
            xt_tiles = {}

            def load_xt(i):
                t = iop.tile([SB, XTW], FP16, tag="xt", name="xt")
                qtr = XTW // 4
                for h in range(4):
                    nc.sync.dma_start(
                        out=t[:, h * qtr:(h + 1) * qtr],
                        in_=xt[:, i * XTW + h * qtr:i * XTW + (h + 1) * qtr])
                xt_tiles[i] = t

            load_xt(0)
            cuf = sp.tile([SB, SB], F32, tag="cuf", name="cuf")
            clf = sp.tile([SB, SB], F32, tag="clf", name="clf")
            czf = sp.tile([SB, 255], F32, tag="czf", name="czf")
            nc.sync.dma_start(out=cuf[:], in_=cu)
            nc.sync.dma_start(out=clf[:], in_=cl)
            nc.sync.dma_start(out=czf[:], in_=cz)
            uh = sp.tile([SB, SB], FP16, tag="uh", name="uh")
            clb = sp.tile([SB, SB], BF16, tag="clb", name="clb")
            czb = sp.tile([SB, 255], BF16, tag="czb", name="czb")


            rgi = pp.tile([128, ST], F32, tag="rgi", name="rgi")
            rser = pp.tile([128, ST], F32, tag="rser", name="rser")

            b_tiles = {}
            BPW = 4096

            def poly_b(j):
                """b = r*(1 - r/2) for 4096-col chunk j (DVE, wide instrs
                so the 4x/2x perf modes engage)."""
                it = xt_tiles[(j * BPW) // XTW]
                off = (j * BPW) % XTW
                tp = bbp.tile([SB, BPW], FP16, tag="tp", name="tp")
                nc.vector.tensor_scalar(tp[:], it[:, off:off + BPW],
                                        -0.5, 1.0, ALU.mult, ALU.add)
                bt = bbp.tile([SB, BPW], FP16, tag="bt", name="bt")
                nc.vector.tensor_tensor(bt[:], it[:, off:off + BPW], tp[:],
                                        ALU.mult)
                b_tiles[j] = bt

            state = {}
            arenas = {}

            def stage_front(k):
                """U'b + sig row + exps for iter k."""
                g = k // 4
                if k % 4 == 0:
                    emr_a = arp.tile([SB, ARW], BF16, tag="emr", name="emr")
                    er_a = arp.tile([SB, ARW], BF16, tag="era", name="era")
                    zt_a = arp.tile([SB, ARW], BF16, tag="zt", name="zt")
                    arenas[g] = (emr_a, er_a, zt_a)
                emr_a, er_a, zt_a = arenas[g]
                aoff = (k % 4) * ZQW
                zq = pp.tile([128, ZQW], F32, tag="px", name="px", bufs=3)
                bt = b_tiles[(k * ZQW) // BPW]
                boff = (k * ZQW) % BPW
                for s in range(2):
                    nc.tensor.matmul(
                        zq[:, s * ST:(s + 1) * ST], uh[:],
                        bt[:, boff + s * ST:boff + (s + 1) * ST])
                nc.scalar.activation(emr_a[:, aoff:aoff + ZQW], zq[:],
                                     AF.Exp, bias=0.0, scale=-BETA)
                nc.scalar.activation(er_a[:, aoff:aoff + ZQW], zq[:],
                                     AF.Exp, bias=0.0, scale=BETA)
                state[k] = (emr_a, er_a, zt_a, aoff, zq)

            def stage_sm(k):
                """Sm + Z for iter k, 2 iters deferred: Sm overwrites the
                px tile (exps long done reading it), so all loop deps except
                exp(k)<-U'b(k) are >=1 iter old and ACT runs gapless."""
                emr_a, er_a, zt_a, aoff, x = state[k]
                for s in range(2):
                    nc.tensor.matmul(
                        x[:, s * ST:(s + 1) * ST], clb[:],
                        emr_a[:, aoff + s * ST:aoff + (s + 1) * ST])
                for s in range(2):
                    nc.vector.scalar_tensor_tensor(
                        zt_a[:, aoff + s * ST:aoff + (s + 1) * ST],
                        x[:, s * ST:(s + 1) * ST], 1.0,
                        er_a[:, aoff + s * ST:aoff + (s + 1) * ST],
                        ALU.bypass, ALU.mult)

            def stage_back(k):
                """rot-gi + rot-ser for iter k (zt ready)."""
                emr_a, er_a, zt_a, aoff, _x = state.pop(k)
                for s in range(2):
                    u = 2 * k + s
                    nc.tensor.matmul(
                        rgi[:], czb[:, 127 - u:255 - u],
                        zt_a[:, aoff + s * ST:aoff + (s + 1) * ST],
                        start=(u == 0), stop=(u == 2 * NIT - 1),
                        skip_group_check=True)
                for s in range(2):
                    u = 2 * k + s
                    nc.tensor.matmul(
                        rser[:], czb[:, 127 - u:255 - u],
                        er_a[:, aoff + s * ST:aoff + (s + 1) * ST],
                        start=(u == 0), stop=(u == 2 * NIT - 1),
                        skip_group_check=True)
                if k % 4 == 3:
                    g = k // 4
                    emr_a2, er_a2, zt_a2 = arenas.pop(g)
                    nc.sync.dma_start(out=o_mr[g:g + 1, :],
                                      in_=emr_a2[127:128, :])
                    nc.sync.dma_start(out=o_er[g:g + 1, :],
                                      in_=er_a2[127:128, :])
                    nc.sync.dma_start(out=o_zp[g:g + 1, :],
                                      in_=zt_a2[127:128, :])

            poly_b(0)
            nc.vector.tensor_copy(uh[:], cuf[:])
            nc.vector.tensor_copy(clb[:], clf[:])
            nc.vector.tensor_copy(czb[:], czf[:])
            for k in range(NIT):
                if (k * ZQW) % XTW == 0:
                    nxt = (k * ZQW) // XTW + 1
                    if nxt < F // XTW:
                        load_xt(nxt)
                if (k * ZQW) % BPW == 0:
                    nxtj = (k * ZQW) // BPW + 1
                    if nxtj < F // BPW:
                        poly_b(nxtj)
                stage_front(k)
                if k >= 5:
                    stage_back(k - 5)
                if k >= 2:
                    stage_sm(k - 2)
            stage_sm(NIT - 2)
            stage_sm(NIT - 1)
            for k in range(NIT - 5, NIT):
                stage_back(k)

            go = sp.tile([128, ST], F32, tag="go", name="go")
            so = sp.tile([128, ST], F32, tag="so", name="so")
            nc.vector.tensor_copy(go[:], rgi[:])
            nc.scalar.activation(so[:], rser[:], AF.Copy, bias=0.0,
                                 scale=1.0)
            nc.sync.dma_start(out=o_gi, in_=go[:])
            nc.sync.dma_start(out=o_sr, in_=so[:])

    nc.compile()
    _built["nc"] = nc
    return nc


def make_consts():
    cu = np.tril(np.ones((SB, SB), np.float32), -1)      # [k, i]: k > i
    cu[:, 127] = 1.0                                      # row 127 -> sig
    cl = np.triu(np.ones((SB, SB), np.float32))          # [k, i]: k <= i
    cz = np.zeros((SB, 255), np.float32)
    cz[:, 127] = 1.0
    return cu, cl, cz


def prep_core(r, c):
    """r [T, B] -> device layout [SB, F] fp16 for core c (path-major)."""
    sub = r[:, c * PL:(c + 1) * PL]                      # [T, PL]
    a = sub.reshape(Q, SB, PL)                            # (q, p, path)
    return np.ascontiguousarray(
        a.transpose(1, 2, 0).reshape(SB, PL * Q)).astype(np.float16)


def host_sig(xth):
    """Replicate the device's per-block sig = sum_p b from the fp16 input
    tile [SB, F]: b = r*(1 - r/2) in fp16, summed over partitions in f32
    (matches the PE's PSUM accumulation to ~1e-7)."""
    r = xth
    t = (r * np.float16(-0.5) + np.float16(1.0)).astype(np.float16)
    b = (r * t).astype(np.float16)
    s = b.astype(np.float32).sum(axis=0, dtype=np.float32)   # [F]
    return s.astype(np.float64).reshape(PL, Q)


def combine(sig, mr, er127, zp, sr, gi):
    """All [PL, Q] f64 -> loss [PL]."""
    SIG = BETA * sig                                      # exact log-domain
    ser_t = sr - er127 + 1.0                              # true sum er
    ep = zp / er127 - mr + 1.0                            # true sum emr
    gi_t = gi - zp + ep
    with np.errstate(divide="ignore", invalid="ignore"):
        l_gi = np.log(gi_t)
        l_em = -SIG + np.log(ser_t)
        l_vend = np.log(ep)
    l_wend = -SIG
    nq = sig.shape[1]
    LG = np.full(sig.shape[0], -np.inf)
    LV = np.full(sig.shape[0], -np.inf)
    for qq in range(nq):
        LG = np.logaddexp(LG, np.logaddexp(l_gi[:, qq], l_em[:, qq] + LV))
        LV = np.logaddexp(l_vend[:, qq], l_wend[:, qq] + LV)
    soft_mdd_log = LG / BETA
    mdd = 1.0 - np.exp(-soft_mdd_log)
    return MDD_LAMBDA * np.maximum(mdd - THRESHOLD, 0.0)


def _f64(a):
    return np.asarray(a, np.float32).astype(np.float64).reshape(PL, Q)


def _run(path_returns, trace=False):
    nc = build()
    cu, cl, cz = make_consts()
    in_maps = [{"xt": prep_core(path_returns, c), "cu": cu, "cl": cl,
                "cz": cz} for c in range(NCORES)]
    res = run_bass_kernel_spmd(nc, in_maps, list(range(NCORES)), trace=trace)
    out = np.empty(B, np.float64)
    for c in range(NCORES):
        r = res.results[c]
        out[c * PL:(c + 1) * PL] = combine(
            host_sig(in_maps[c]["xt"]), _f64(r["mr"]), _f64(r["er"]),
            _f64(r["zp"]), _f64(r["sr"]), _f64(r["gi"]))
    return out.astype(np.float32), res


def kernel(path_returns):
    out, _ = _run(path_returns)
    return out


# revision 15
# speedup vs baseline: 1.0836x; 1.0836x over previous
"""Trainium2 Bass kernel for SoftPathMDDLoss — PE-bilinear, ACT-lean.

Math: per 128-step subblock (time on partitions, (path, subblock)
path-major on the free axis), with strict suffix sums zr' = U'.b and
host f64 cross-block composition.  Engine assignment from measured
TRN2 rates:
  - host pre-casts the input to fp16 (halves DMA-in to 16 MiB/core).
  - Ln(1+r) ~= r*(1 - r/2): 2-op fp16 polynomial on DVE at [128,4096]
    granularity where the 4x/2x perf modes engage (r^3/3 < 3e-6 is far
    below the fp16 quantisation already present).  ACT therefore runs
    only the two Exp passes (bf16 out, no cast passes anywhere).
  - sig (per-block sum of b) is not exported: the host replicates the
    fp16 polynomial + f32 partition sum bit-closely (~1e-7), which is
    exact enough for the log-domain corrections.
  - per-unit row exports (emr127 / er127 / z127) collect in bf16 arena
    tiles (3 bufs) and DMA once per 8 units; ~85 dma_starts total.
  - PE: 4 passes (U'b cumsum, Sm = L emr, rotating one-hot partition
    sums of Z = er*Sm -> gi and of er -> ser), 512-col matmuls, fp16/
    bf16 moving at 1 cyc/row; sustained-fed PE holds the 2.4 GHz
    p-state.
Pipeline: per 1024-col iter k issue U'b(k)+exps(k), rot(k-3), Sm/Z(k-1);
PSUM = zq[1024]x2 + smt[1024] + rgi + rser = 8 banks.
Measured: ~192.6 us on 8 cores (baseline 226.9), rel err 2.5e-3.
"""
import numpy as np

import concourse.bacc as bacc
import concourse.mybir as mybir
import concourse.tile as tile
from concourse.bass_utils import run_bass_kernel_spmd

T, B = 65536, 1024
NCORES = 8
PL = 128                  # paths per core (unit u <-> path u)
SB = 128                  # subblock length = partitions
Q = T // SB               # 512 subblocks per path
F = PL * Q                # 65536 free columns per core
ST = 512                  # unit width = one psum bank
ZQW = 1024                # zq tile width (2 units, one iter)
XTW = 8192                # input tile width
ARW = 4096                # arena width (8 units = 4 iters)
NGR = F // ARW            # 16 arena groups
NIT = F // ZQW            # 64 iters
BETA = 40.0
THRESHOLD = 0.12
MDD_LAMBDA = 5.0

F32 = mybir.dt.float32
BF16 = mybir.dt.bfloat16
FP16 = mybir.dt.float16
AF = mybir.ActivationFunctionType
ALU = mybir.AluOpType

_built = {}


def build():
    if "nc" in _built:
        return _built["nc"]
    nc = bacc.Bacc("TRN2", target_bir_lowering=False, debug=False)
    xt = nc.dram_tensor("xt", [SB, F], FP16, kind="ExternalInput").ap()
    cu = nc.dram_tensor("cu", [SB, SB], F32, kind="ExternalInput").ap()
    cl = nc.dram_tensor("cl", [SB, SB], F32, kind="ExternalInput").ap()
    cz = nc.dram_tensor("cz", [SB, 255], F32, kind="ExternalInput").ap()
    o_mr = nc.dram_tensor("mr", [NGR, ARW], BF16, kind="ExternalOutput").ap()
    o_er = nc.dram_tensor("er", [NGR, ARW], BF16, kind="ExternalOutput").ap()
    o_zp = nc.dram_tensor("zp", [NGR, ARW], BF16, kind="ExternalOutput").ap()
    o_sr = nc.dram_tensor("sr", [128, ST], F32, kind="ExternalOutput").ap()
    o_gi = nc.dram_tensor("gi", [128, ST], F32, kind="ExternalOutput").ap()

    with tile.TileContext(nc) as tc:
        with tc.tile_pool(name="io", bufs=2) as iop, \
             tc.tile_pool(name="bb", bufs=2) as bbp, \
             tc.tile_pool(name="ar", bufs=3) as arp, \
             tc.tile_pool(name="st", bufs=1) as sp, \
             tc.tile_pool(name="ps", bufs=1,
                          space=bacc.bass.MemorySpace.PSUM) as pp:
            xt_tiles = {}

            def load_xt(i):
                t = iop.tile([SB, XTW], FP16, tag="xt", name="xt")
                qtr = XTW // 4
                for h in range(4):
                    nc.sync.dma_start(
                        out=t[:, h * qtr:(h + 1) * qtr],
                        in_=xt[:, i * XTW + h * qtr:i * XTW + (h + 1) * qtr])
                xt_tiles[i] = t

            load_xt(0)
            cuf = sp.tile([SB, SB], F32, tag="cuf", name="cuf")
            clf = sp.tile([SB, SB], F32, tag="clf", name="clf")
            czf = sp.tile([SB, 255], F32, tag="czf", name="czf")
            nc.sync.dma_start(out=cuf[:], in_=cu)
            nc.sync.dma_start(out=clf[:], in_=cl)
            nc.sync.dma_start(out=czf[:], in_=cz)
            uh = sp.tile([SB, SB], FP16, tag="uh", name="uh")
            clb = sp.tile([SB, SB], BF16, tag="clb", name="clb")
            czb = sp.tile([SB, 255], BF16, tag="czb", name="czb")


            rgi = pp.tile([128, ST], F32, tag="rgi", name="rgi")
            rser = pp.tile([128, ST], F32, tag="rser", name="rser")

            b_tiles = {}
            BPW = 4096

            def poly_b(j):
                """b = r*(1 - r/2) for 4096-col chunk j (DVE, wide instrs
                so the 4x/2x perf modes engage)."""
                it = xt_tiles[(j * BPW) // XTW]
                off = (j * BPW) % XTW
                tp = bbp.tile([SB, BPW], FP16, tag="tp", name="tp")
                nc.vector.tensor_scalar(tp[:], it[:, off:off + BPW],
                                        -0.5, 1.0, ALU.mult, ALU.add)
                bt = bbp.tile([SB, BPW], FP16, tag="bt", name="bt")
                nc.vector.tensor_tensor(bt[:], it[:, off:off + BPW], tp[:],
                                        ALU.mult)
                b_tiles[j] = bt

            state = {}
            arenas = {}

            def stage_front(k):
                """U'b + sig row + exps for iter k."""
                g = k // 4
                if k % 4 == 0:
                    emr_a = arp.tile([SB, ARW], BF16, tag="emr", name="emr")
                    er_a = arp.tile([SB, ARW], BF16, tag="era", name="era")
                    zt_a = arp.tile([SB, ARW], BF16, tag="zt", name="zt")
                    arenas[g] = (emr_a, er_a, zt_a)
                emr_a, er_a, zt_a = arenas[g]
                aoff = (k % 4) * ZQW
                zq = pp.tile([128, ZQW], F32, tag="px", name="px", bufs=3)
                bt = b_tiles[(k * ZQW) // BPW]
                boff = (k * ZQW) % BPW
                for s in range(2):
                    nc.tensor.matmul(
                        zq[:, s * ST:(s + 1) * ST], uh[:],
                        bt[:, boff + s * ST:boff + (s + 1) * ST])
                nc.scalar.activation(emr_a[:, aoff:aoff + ZQW], zq[:],
                                     AF.Exp, bias=0.0, scale=-BETA)
                nc.scalar.activation(er_a[:, aoff:aoff + ZQW], zq[:],
                                     AF.Exp, bias=0.0, scale=BETA)
                state[k] = (emr_a, er_a, zt_a, aoff, zq)

            def stage_sm(k):
                """Sm + Z for iter k, 2 iters deferred: Sm overwrites the
                px tile (exps long done reading it), so all loop deps except
                exp(k)<-U'b(k) are >=1 iter old and ACT runs gapless."""
                emr_a, er_a, zt_a, aoff, x = state[k]
                for s in range(2):
                    nc.tensor.matmul(
                        x[:, s * ST:(s + 1) * ST], clb[:],
                        emr_a[:, aoff + s * ST:aoff + (s + 1) * ST])
                nc.vector.scalar_tensor_tensor(
                    zt_a[:, aoff:aoff + ZQW], x[:], 1.0,
                    er_a[:, aoff:aoff + ZQW], ALU.bypass, ALU.mult)

            def stage_back(k):
                """rot-gi + rot-ser for iter k (zt ready)."""
                emr_a, er_a, zt_a, aoff, _x = state.pop(k)
                for s in range(2):
                    u = 2 * k + s
                    nc.tensor.matmul(
                        rgi[:], czb[:, 127 - u:255 - u],
                        zt_a[:, aoff + s * ST:aoff + (s + 1) * ST],
                        start=(u == 0), stop=(u == 2 * NIT - 1),
                        skip_group_check=True)
                for s in range(2):
                    u = 2 * k + s
                    nc.tensor.matmul(
                        rser[:], czb[:, 127 - u:255 - u],
                        er_a[:, aoff + s * ST:aoff + (s + 1) * ST],
                        start=(u == 0), stop=(u == 2 * NIT - 1),
                        skip_group_check=True)
                if k % 4 == 3:
                    g = k // 4
                    emr_a2, er_a2, zt_a2 = arenas.pop(g)
                    nc.sync.dma_start(out=o_mr[g:g + 1, :],
                                      in_=emr_a2[127:128, :])
                    nc.sync.dma_start(out=o_er[g:g + 1, :],
                                      in_=er_a2[127:128, :])
                    nc.sync.dma_start(out=o_zp[g:g + 1, :],
                                      in_=zt_a2[127:128, :])

            poly_b(0)
            nc.vector.tensor_copy(uh[:], cuf[:])
            nc.vector.tensor_copy(clb[:], clf[:])
            nc.vector.tensor_copy(czb[:], czf[:])
            for k in range(NIT):
                if (k * ZQW) % XTW == 0:
                    nxt = (k * ZQW) // XTW + 1
                    if nxt < F // XTW:
                        load_xt(nxt)
                if (k * ZQW) % BPW == 0:
                    nxtj = (k * ZQW) // BPW + 1
                    if nxtj < F // BPW:
                        poly_b(nxtj)
                stage_front(k)
                if k >= 5:
                    stage_back(k - 5)
                if k >= 2:
                    stage_sm(k - 2)
            stage_sm(NIT - 2)
            stage_sm(NIT - 1)
            for k in range(NIT - 5, NIT):
                stage_back(k)

            go = sp.tile([128, ST], F32, tag="go", name="go")
            so = sp.tile([128, ST], F32, tag="so", name="so")
            nc.vector.tensor_copy(go[:], rgi[:])
            nc.scalar.activation(so[:], rser[:], AF.Copy, bias=0.0,
                                 scale=1.0)
            nc.sync.dma_start(out=o_gi, in_=go[:])
            nc.sync.dma_start(out=o_sr, in_=so[:])

    nc.compile()
    _built["nc"] = nc
    return nc


def make_consts():
    cu = np.tril(np.ones((SB, SB), np.float32), -1)      # [k, i]: k > i
    cu[:, 127] = 1.0                                      # row 127 -> sig
    cl = np.triu(np.ones((SB, SB), np.float32))          # [k, i]: k <= i
    cz = np.zeros((SB, 255), np.float32)
    cz[:, 127] = 1.0
    return cu, cl, cz


def prep_core(r, c):
    """r [T, B] -> device layout [SB, F] fp16 for core c (path-major)."""
    sub = r[:, c * PL:(c + 1) * PL]                      # [T, PL]
    a = sub.reshape(Q, SB, PL)                            # (q, p, path)
    return np.ascontiguousarray(
        a.transpose(1, 2, 0).reshape(SB, PL * Q)).astype(np.float16)


def host_sig(xth):
    """Replicate the device's per-block sig = sum_p b from the fp16 input
    tile [SB, F]: b = r*(1 - r/2) in fp16, summed over partitions in f32
    (matches the PE's PSUM accumulation to ~1e-7)."""
    r = xth
    t = (r * np.float16(-0.5) + np.float16(1.0)).astype(np.float16)
    b = (r * t).astype(np.float16)
    s = b.astype(np.float32).sum(axis=0, dtype=np.float32)   # [F]
    return s.astype(np.float64).reshape(PL, Q)


def combine(sig, mr, er127, zp, sr, gi):
    """All [PL, Q] f64 -> loss [PL]."""
    SIG = BETA * sig                                      # exact log-domain
    ser_t = sr - er127 + 1.0                              # true sum er
    ep = zp / er127 - mr + 1.0                            # true sum emr
    gi_t = gi - zp + ep
    with np.errstate(divide="ignore", invalid="ignore"):
        l_gi = np.log(gi_t)
        l_em = -SIG + np.log(ser_t)
        l_vend = np.log(ep)
    l_wend = -SIG
    nq = sig.shape[1]
    LG = np.full(sig.shape[0], -np.inf)
    LV = np.full(sig.shape[0], -np.inf)
    for qq in range(nq):
        LG = np.logaddexp(LG, np.logaddexp(l_gi[:, qq], l_em[:, qq] + LV))
        LV = np.logaddexp(l_vend[:, qq], l_wend[:, qq] + LV)
    soft_mdd_log = LG / BETA
    mdd = 1.0 - np.exp(-soft_mdd_log)
    return MDD_LAMBDA * np.maximum(mdd - THRESHOLD, 0.0)


def _f64(a):
    return np.asarray(a, np.float32).astype(np.float64).reshape(PL, Q)


def _run(path_returns, trace=False):
    nc = build()
    cu, cl, cz = make_consts()
    in_maps = [{"xt": prep_core(path_returns, c), "cu": cu, "cl": cl,
                "cz": cz} for c in range(NCORES)]
    res = run_bass_kernel_spmd(nc, in_maps, list(range(NCORES)), trace=trace)
    out = np.empty(B, np.float64)
    for c in range(NCORES):
        r = res.results[c]
        out[c * PL:(c + 1) * PL] = combine(
            host_sig(in_maps[c]["xt"]), _f64(r["mr"]), _f64(r["er"]),
            _f64(r["zp"]), _f64(r["sr"]), _f64(r["gi"]))
    return out.astype(np.float32), res


def kernel(path_returns):
    out, _ = _run(path_returns)
    return out


# revision 16
# speedup vs baseline: 1.1296x; 1.0425x over previous
"""Trainium2 Bass kernel for SoftPathMDDLoss — PE-bilinear, ACT-lean.

Math: per 128-step subblock (time on partitions, (path, subblock)
path-major on the free axis), with strict suffix sums zr' = U'.b and
host f64 cross-block composition.  Engine assignment from measured
TRN2 rates:
  - host pre-casts the input to fp16 (halves DMA-in to 16 MiB/core).
  - Ln(1+r) ~= r*(1 - r/2): 2-op fp16 polynomial on DVE at [128,4096]
    granularity where the 4x/2x perf modes engage (r^3/3 < 3e-6 is far
    below the fp16 quantisation already present).  ACT therefore runs
    only the two Exp passes (bf16 out, no cast passes anywhere).
  - sig (per-block sum of b) is not exported: the host replicates the
    fp16 polynomial + f32 partition sum bit-closely (~1e-7), which is
    exact enough for the log-domain corrections.
  - per-unit row exports (emr127 / er127 / z127) collect in bf16 arena
    tiles (3 bufs) and DMA once per 8 units; ~85 dma_starts total.
  - PE: 4 passes (U'b cumsum, Sm = L emr, rotating one-hot partition
    sums of Z = er*Sm -> gi and of er -> ser), 512-col matmuls, fp16/
    bf16 moving at 1 cyc/row; sustained-fed PE holds the 2.4 GHz
    p-state.
Pipeline: per 1024-col iter k issue U'b(k)+exps(k), rot(k-3), Sm/Z(k-1);
PSUM = zq[1024]x2 + smt[1024] + rgi + rser = 8 banks.
Measured: ~192.6 us on 8 cores (baseline 226.9), rel err 2.5e-3.
"""
import numpy as np

import concourse.bacc as bacc
import concourse.mybir as mybir
import concourse.tile as tile
from concourse.bass_utils import run_bass_kernel_spmd

T, B = 65536, 1024
NCORES = 8
PL = 128                  # paths per core (unit u <-> path u)
SB = 128                  # subblock length = partitions
Q = T // SB               # 512 subblocks per path
F = PL * Q                # 65536 free columns per core
ST = 512                  # unit width = one psum bank
ZQW = 1024                # zq tile width (2 units, one iter)
XTW = 8192                # input tile width
ARW = 4096                # arena width (8 units = 4 iters)
NGR = F // ARW            # 16 arena groups
NIT = F // ZQW            # 64 iters
BETA = 40.0
THRESHOLD = 0.12
MDD_LAMBDA = 5.0

F32 = mybir.dt.float32
BF16 = mybir.dt.bfloat16
FP16 = mybir.dt.float16
AF = mybir.ActivationFunctionType
ALU = mybir.AluOpType

_built = {}


def build():
    if "nc" in _built:
        return _built["nc"]
    nc = bacc.Bacc("TRN2", target_bir_lowering=False, debug=False)
    xt = nc.dram_tensor("xt", [SB, F], FP16, kind="ExternalInput").ap()
    cu = nc.dram_tensor("cu", [SB, SB], F32, kind="ExternalInput").ap()
    cl = nc.dram_tensor("cl", [SB, SB], F32, kind="ExternalInput").ap()
    cz = nc.dram_tensor("cz", [SB, 255], F32, kind="ExternalInput").ap()
    o_mr = nc.dram_tensor("mr", [NGR, ARW], BF16, kind="ExternalOutput").ap()
    o_er = nc.dram_tensor("er", [NGR, ARW], BF16, kind="ExternalOutput").ap()
    o_zp = nc.dram_tensor("zp", [NGR, ARW], BF16, kind="ExternalOutput").ap()
    o_sr = nc.dram_tensor("sr", [128, ST], F32, kind="ExternalOutput").ap()
    o_gi = nc.dram_tensor("gi", [128, ST], F32, kind="ExternalOutput").ap()

    with tile.TileContext(nc) as tc:
        with tc.tile_pool(name="io", bufs=2) as iop, \
             tc.tile_pool(name="bb", bufs=2) as bbp, \
             tc.tile_pool(name="ar", bufs=3) as arp, \
             tc.tile_pool(name="st", bufs=1) as sp, \
             tc.tile_pool(name="ps", bufs=1,
                          space=bacc.bass.MemorySpace.PSUM) as pp:
            xt_tiles = {}

            def load_xt(i):
                t = iop.tile([SB, XTW], FP16, tag="xt", name="xt")
                qtr = XTW // 4
                for h in range(4):
                    nc.sync.dma_start(
                        out=t[:, h * qtr:(h + 1) * qtr],
                        in_=xt[:, i * XTW + h * qtr:i * XTW + (h + 1) * qtr])
                xt_tiles[i] = t

            load_xt(0)
            cuf = sp.tile([SB, SB], F32, tag="cuf", name="cuf")
            clf = sp.tile([SB, SB], F32, tag="clf", name="clf")
            czf = sp.tile([SB, 255], F32, tag="czf", name="czf")
            nc.sync.dma_start(out=cuf[:], in_=cu)
            nc.sync.dma_start(out=clf[:], in_=cl)
            nc.sync.dma_start(out=czf[:], in_=cz)
            uh = sp.tile([SB, SB], FP16, tag="uh", name="uh")
            clb = sp.tile([SB, SB], BF16, tag="clb", name="clb")
            czb = sp.tile([SB, 255], BF16, tag="czb", name="czb")


            rgi = pp.tile([128, ST], F32, tag="rgi", name="rgi")
            rser = pp.tile([128, ST], F32, tag="rser", name="rser")

            b_tiles = {}
            BPW = 4096

            def poly_b(j):
                """b = r*(1 - r/2) for 4096-col chunk j (DVE, wide instrs
                so the 4x/2x perf modes engage)."""
                it = xt_tiles[(j * BPW) // XTW]
                off = (j * BPW) % XTW
                tp = bbp.tile([SB, BPW], FP16, tag="tp", name="tp")
                nc.vector.tensor_scalar(tp[:], it[:, off:off + BPW],
                                        -0.5, 1.0, ALU.mult, ALU.add)
                bt = bbp.tile([SB, BPW], FP16, tag="bt", name="bt")
                nc.vector.tensor_tensor(bt[:], it[:, off:off + BPW], tp[:],
                                        ALU.mult)
                b_tiles[j] = bt

            state = {}
            arenas = {}

            def stage_front(k):
                """U'b + sig row + exps for iter k."""
                g = k // 4
                if k % 4 == 0:
                    emr_a = arp.tile([SB, ARW], BF16, tag="emr", name="emr")
                    er_a = arp.tile([SB, ARW], BF16, tag="era", name="era")
                    zt_a = arp.tile([SB, ARW], BF16, tag="zt", name="zt")
                    arenas[g] = (emr_a, er_a, zt_a)
                emr_a, er_a, zt_a = arenas[g]
                aoff = (k % 4) * ZQW
                zq = pp.tile([128, ZQW], F32, tag="px", name="px", bufs=3)
                bt = b_tiles[(k * ZQW) // BPW]
                boff = (k * ZQW) % BPW
                for s in range(2):
                    nc.tensor.matmul(
                        zq[:, s * ST:(s + 1) * ST], uh[:],
                        bt[:, boff + s * ST:boff + (s + 1) * ST])
                nc.scalar.activation(emr_a[:, aoff:aoff + ZQW], zq[:],
                                     AF.Exp, bias=0.0, scale=-BETA)
                nc.scalar.activation(er_a[:, aoff:aoff + ZQW], zq[:],
                                     AF.Exp, bias=0.0, scale=BETA)
                state[k] = (emr_a, er_a, zt_a, aoff, zq)

            def stage_sm(k):
                """Sm + Z for iter k, 2 iters deferred: Sm overwrites the
                px tile (exps long done reading it), so all loop deps except
                exp(k)<-U'b(k) are >=1 iter old and ACT runs gapless."""
                emr_a, er_a, zt_a, aoff, x = state[k]
                for s in range(2):
                    nc.tensor.matmul(
                        x[:, s * ST:(s + 1) * ST], clb[:],
                        emr_a[:, aoff + s * ST:aoff + (s + 1) * ST])
                nc.vector.scalar_tensor_tensor(
                    zt_a[:, aoff:aoff + ZQW], x[:], 1.0,
                    er_a[:, aoff:aoff + ZQW], ALU.bypass, ALU.mult)

            def stage_back(k):
                """rot-gi + rot-ser for iter k (zt ready)."""
                emr_a, er_a, zt_a, aoff, _x = state.pop(k)
                for s in range(2):
                    u = 2 * k + s
                    nc.tensor.matmul(
                        rgi[:], czb[:, 127 - u:255 - u],
                        zt_a[:, aoff + s * ST:aoff + (s + 1) * ST],
                        start=(u == 0), stop=(u == 2 * NIT - 1),
                        skip_group_check=True)
                for s in range(2):
                    u = 2 * k + s
                    nc.tensor.matmul(
                        rser[:], czb[:, 127 - u:255 - u],
                        er_a[:, aoff + s * ST:aoff + (s + 1) * ST],
                        start=(u == 0), stop=(u == 2 * NIT - 1),
                        skip_group_check=True)
                if k % 4 == 3:
                    g = k // 4
                    emr_a2, er_a2, zt_a2 = arenas.pop(g)
                    nc.sync.dma_start(out=o_mr[g:g + 1, :],
                                      in_=emr_a2[127:128, :])
                    nc.sync.dma_start(out=o_er[g:g + 1, :],
                                      in_=er_a2[127:128, :])
                    nc.sync.dma_start(out=o_zp[g:g + 1, :],
                                      in_=zt_a2[127:128, :])

            poly_b(0)
            nc.vector.tensor_copy(uh[:], cuf[:])
            nc.vector.tensor_copy(clb[:], clf[:])
            nc.vector.tensor_copy(czb[:], czf[:])
            for k in range(NIT):
                if (k * ZQW) % XTW == 0:
                    nxt = (k * ZQW) // XTW + 1
                    if nxt < F // XTW:
                        load_xt(nxt)
                if (k * ZQW) % BPW == 0:
                    nxtj = (k * ZQW) // BPW + 1
                    if nxtj < F // BPW:
                        poly_b(nxtj)
                if k >= 2:
                    stage_sm(k - 2)
                stage_front(k)
                if k >= 5:
                    stage_back(k - 5)
            stage_sm(NIT - 2)
            stage_sm(NIT - 1)
            for k in range(NIT - 5, NIT):
                stage_back(k)

            go = sp.tile([128, ST], F32, tag="go", name="go")
            so = sp.tile([128, ST], F32, tag="so", name="so")
            nc.vector.tensor_copy(go[:], rgi[:])
            nc.scalar.activation(so[:], rser[:], AF.Copy, bias=0.0,
                                 scale=1.0)
            nc.sync.dma_start(out=o_gi, in_=go[:])
            nc.sync.dma_start(out=o_sr, in_=so[:])

    nc.compile()
    _built["nc"] = nc
    return nc


def make_consts():
    cu = np.tril(np.ones((SB, SB), np.float32), -1)      # [k, i]: k > i
    cu[:, 127] = 1.0                                      # row 127 -> sig
    cl = np.triu(np.ones((SB, SB), np.float32))          # [k, i]: k <= i
    cz = np.zeros((SB, 255), np.float32)
    cz[:, 127] = 1.0
    return cu, cl, cz


def prep_core(r, c):
    """r [T, B] -> device layout [SB, F] fp16 for core c (path-major)."""
    sub = r[:, c * PL:(c + 1) * PL]                      # [T, PL]
    a = sub.reshape(Q, SB, PL)                            # (q, p, path)
    return np.ascontiguousarray(
        a.transpose(1, 2, 0).reshape(SB, PL * Q)).astype(np.float16)


def host_sig(xth):
    """Replicate the device's per-block sig = sum_p b from the fp16 input
    tile [SB, F]: b = r*(1 - r/2) in fp16, summed over partitions in f32
    (matches the PE's PSUM accumulation to ~1e-7)."""
    r = xth
    t = (r * np.float16(-0.5) + np.float16(1.0)).astype(np.float16)
    b = (r * t).astype(np.float16)
    s = b.astype(np.float32).sum(axis=0, dtype=np.float32)   # [F]
    return s.astype(np.float64).reshape(PL, Q)


def combine(sig, mr, er127, zp, sr, gi):
    """All [PL, Q] f64 -> loss [PL]."""
    SIG = BETA * sig                                      # exact log-domain
    ser_t = sr - er127 + 1.0                              # true sum er
    ep = zp / er127 - mr + 1.0                            # true sum emr
    gi_t = gi - zp + ep
    with np.errstate(divide="ignore", invalid="ignore"):
        l_gi = np.log(gi_t)
        l_em = -SIG + np.log(ser_t)
        l_vend = np.log(ep)
    l_wend = -SIG
    nq = sig.shape[1]
    LG = np.full(sig.shape[0], -np.inf)
    LV = np.full(sig.shape[0], -np.inf)
    for qq in range(nq):
        LG = np.logaddexp(LG, np.logaddexp(l_gi[:, qq], l_em[:, qq] + LV))
        LV = np.logaddexp(l_vend[:, qq], l_wend[:, qq] + LV)
    soft_mdd_log = LG / BETA
    mdd = 1.0 - np.exp(-soft_mdd_log)
    return MDD_LAMBDA * np.maximum(mdd - THRESHOLD, 0.0)


def _f64(a):
    return np.asarray(a, np.float32).astype(np.float64).reshape(PL, Q)


def _run(path_returns, trace=False):
    nc = build()
    cu, cl, cz = make_consts()
    in_maps = [{"xt": prep_core(path_returns, c), "cu": cu, "cl": cl,
                "cz": cz} for c in range(NCORES)]
    res = run_bass_kernel_spmd(nc, in_maps, list(range(NCORES)), trace=trace)
    out = np.empty(B, np.float64)
    for c in range(NCORES):
        r = res.results[c]
        out[c * PL:(c + 1) * PL] = combine(
            host_sig(in_maps[c]["xt"]), _f64(r["mr"]), _f64(r["er"]),
            _f64(r["zp"]), _f64(r["sr"]), _f64(r["gi"]))
    return out.astype(np.float32), res


def kernel(path_returns):
    out, _ = _run(path_returns)
    return out
